# revision 3
# baseline (speedup 1.0000x reference)
"""Trainium2 Bass kernel for windowed multi-head attention (ClassicAttention).

Shapes (hardcoded per spec): x (1024, 68, 768), pe (128, 768), mask zeros.
Data-parallel over 8 NeuronCores on the leading window axis.

v2: bf16 matmul operands everywhere (fp32 HIGH mode ran at 4 passes/row),
pe folded into x on host, softmax normalization applied at the AT stage
(sum broadcast + wide reciprocal instead of a [1,408] DVE reciprocal).
"""

import os
import sys

for _p in (
    "/root/.axon_site",
    "/root/.axon_site/_ro/trn_rl_repo",
    "/root/.axon_site/_ro/pypackages",
    "/opt/trn_rl_repo",
):
    if os.path.isdir(_p) and _p not in sys.path:
        sys.path.append(_p)

import numpy as np
import ml_dtypes

import concourse.bass as bass
import concourse.mybir as mybir
import concourse.tile as tile
from concourse import bacc
from concourse.bass_utils import run_bass_kernel_spmd

F32 = mybir.dt.float32
BF16 = mybir.dt.bfloat16
NP_BF16 = np.dtype(ml_dtypes.bfloat16)
AFT = mybir.ActivationFunctionType

NCORES = 8
B_, N, C = 1024, 68, 768
H, HD = 12, 64
N_VTS = 4
KT = C // 128            # 6 contraction tiles of 128
BL = B_ // NCORES        # 128 windows per core
G = 4                    # windows per group (fp32 PSUM 4*68=272 <= 512)
NG = BL // G             # 32 groups
FD = G * N               # 272

_CACHE = {}


def _build_nc():
    nc = bacc.Bacc(trn_type="TRN2", target_bir_lowering=False, debug=False)

    xt_d = nc.dram_tensor("xt", [128, KT, BL, N], BF16, kind="ExternalInput")
    w1_d = nc.dram_tensor("w1", [128, 12, KT, 128], BF16, kind="ExternalInput")
    w2_d = nc.dram_tensor("w2", [128, KT, C], BF16, kind="ExternalInput")
    wp_d = nc.dram_tensor("wp", [128, KT, KT, 128], BF16, kind="ExternalInput")
    ones68_d = nc.dram_tensor("ones68", [N, 1], BF16, kind="ExternalInput")
    out_d = nc.dram_tensor("outt", [128, KT, BL, N], F32, kind="ExternalOutput")

    with tile.TileContext(nc) as tc:
        with (
            tc.tile_pool(name="wgt", bufs=1) as wp_pool,
            tc.tile_pool(name="xp", bufs=2) as xp,
            tc.tile_pool(name="qkp", bufs=2) as qkp,
            tc.tile_pool(name="vp", bufs=2) as vp,
            tc.tile_pool(name="esp", bufs=2) as esp,
            tc.tile_pool(name="rp", bufs=2) as rp,
            tc.tile_pool(name="atp", bufs=2) as atp,
            tc.tile_pool(name="pbig", bufs=2, space="PSUM") as pbig,
            tc.tile_pool(name="ppv", bufs=1, space="PSUM") as ppv,
            tc.tile_pool(name="psc", bufs=2, space="PSUM") as psc,
            tc.tile_pool(name="ps1", bufs=1, space="PSUM") as ps1p,
            tc.tile_pool(name="pav", bufs=1, space="PSUM") as pavp,
        ):
            W1s = wp_pool.tile([128, 12, KT, 128], BF16)
            W2s = wp_pool.tile([128, KT, C], BF16)
            WPs = wp_pool.tile([128, KT, KT, 128], BF16)
            ONES68s = wp_pool.tile([N, 1], BF16)
            nc.sync.dma_start(W1s[:], w1_d.ap())
            nc.sync.dma_start(W2s[:], w2_d.ap())
            nc.sync.dma_start(WPs[:], wp_d.ap())
            nc.sync.dma_start(ONES68s[:], ones68_d.ap())

            for g in range(NG):
                gsl = slice(G * g, G * (g + 1))
                XT = xp.tile([128, KT, G, N], BF16, tag="xt")
                nc.sync.dma_start(XT[:], xt_d.ap()[:, :, gsl, :])

                # ---- q,k in transposed layout: QKT[p, j, w, t] (j<6: q, j>=6: k)
                QKT = qkp.tile([128, 12, G, N], BF16, tag="qkt")
                for j in range(12):
                    pq = pbig.tile([128, FD], F32, tag="big")
                    for k in range(KT):
                        nc.tensor.matmul(
                            pq[:], W1s[:, j, k, :], XT[:, k, :, :],
                            start=(k == 0), stop=(k == KT - 1),
                        )
                    dst = QKT[:, j, :, :]
                    src = pq.rearrange("p (a b) -> p a b", a=G)
                    if j % 2 == 0:
                        nc.vector.tensor_copy(dst, src)
                    else:
                        nc.scalar.copy(dst, src)

                # ---- v in direct layout: V[t, w, o]
                V = vp.tile([N, G, C], BF16, tag="v")
                for w in range(G):
                    pv = ppv.tile([N, C], F32, tag="pv")
                    for hs in (slice(0, 512), slice(512, C)):
                        for k in range(KT):
                            nc.tensor.matmul(
                                pv[:, hs], XT[:, k, w, :], W2s[:, k, hs],
                                start=(k == 0), stop=(k == KT - 1),
                            )
                    nc.scalar.copy(V[:, w, :], pv[:])

                AT = atp.tile([128, KT, G, N], BF16, tag="at")

                def qk_exp(w):
                    # ES slot s = 6*half + hh holds head h = 2*hh + half, so
                    # each PSUM bank sees a single PE row-group (HW hangs on
                    # mixed-row-group matmuls into one bank).
                    ES = esp.tile([N, H, N], BF16, tag="es")
                    for half in range(2):
                        sc = psc.tile([N, 6, N], F32, tag="sc")
                        p0 = 64 * half
                        for hh in range(6):
                            nc.tensor.matmul(
                                sc[:, hh, :],
                                QKT[p0:p0 + 64, 6 + hh, w, :],
                                QKT[p0:p0 + 64, hh, w, :],
                                start=True, stop=True, skip_group_check=True,
                            )
                        nc.scalar.activation(
                            ES[:, 6 * half:6 * half + 6, :], sc[:], AFT.Exp
                        )
                    return ES

                def sums_av(w, ES):
                    # denominators: colsums of exp(S^T) per (slot, n)
                    S1 = rp.tile([1, 2, 408], F32, tag="s1s")
                    for half in range(2):
                        s1 = ps1p.tile([1, 408], F32, tag="s1")
                        nc.tensor.matmul(
                            s1[:], ONES68s[:],
                            ES[:, 6 * half:6 * half + 6, :],
                            start=True, stop=True,
                        )
                        nc.scalar.copy(S1[:, half, :], s1[:])
                    # broadcast sums over partitions: RB[64a+d, s2, n] =
                    # S1[0, a, s2*68+n]  (head h = 2*s2 + a)
                    RB = rp.tile([128, KT, N], F32, tag="rb")
                    for a in range(2):
                        s1h = S1[:, a, :]
                        src = bass.AP(
                            tensor=s1h.tensor, offset=s1h.offset,
                            ap=[list(s1h.ap[0]), [0, 64], [1, 408]],
                        )
                        nc.sync.dma_start(RB[64 * a:64 * a + 64, :, :], src)
                    RBr = rp.tile([128, KT, N], F32, tag="rbr")
                    nc.vector.reciprocal(RBr[:], RB[:])

                    # unnormalized attn @ v, normalized on PSUM->SBUF copy
                    pa = pavp.tile([128, KT, N], F32, tag="pa")
                    for s in range(H):
                        h = 2 * (s % 6) + (s // 6)  # head held in ES slot s
                        nc.tensor.matmul(
                            pa[64 * (s // 6):64 * (s // 6) + 64, s % 6, :],
                            V[:, w, 64 * h:64 * (h + 1)],
                            ES[:, s, :],
                            start=True, stop=True, skip_group_check=True,
                        )
                    nc.vector.tensor_mul(AT[:, :, w, :], pa[:], RBr[:])

                ess = {0: qk_exp(0)}
                for w in range(1, G):
                    ess[w] = qk_exp(w)
                    sums_av(w - 1, ess.pop(w - 1))
                sums_av(G - 1, ess.pop(G - 1))

                # ---- proj: out^T[o, w, t] from PSUM via SBUF to DRAM
                for j in range(KT):
                    po = pbig.tile([128, FD], F32, tag="big")
                    for kt in range(KT):
                        nc.tensor.matmul(
                            po[:], WPs[:, j, kt, :], AT[:, kt, :, :],
                            start=(kt == 0), stop=(kt == KT - 1),
                        )
                    OT = atp.tile([128, G, N], F32, tag="ot")
                    src = po.rearrange("p (a b) -> p a b", a=G)
                    if j % 2 == 0:
                        nc.vector.tensor_copy(OT[:], src)
                    else:
                        nc.scalar.copy(OT[:], src)
                    nc.sync.dma_start(out_d.ap()[:, j, gsl, :], OT[:])

    nc.compile()
    return nc


def _host_prep(x, pe, w_qkv, b_qkv, w_proj, b_proj):
    f = np.float32
    x = np.asarray(x, f)
    pe = np.asarray(pe, f)
    w_qkv = np.asarray(w_qkv, f)
    w_proj = np.asarray(w_proj, f)

    scale = f(HD ** -0.5)
    n_ = N - N_VTS
    strt = pe.shape[0] // 2 - n_ // 2

    # fold pe into x (biases are zero by problem construction)
    xp = x.copy()
    xp[:, N_VTS:, :] += pe[strt:strt + n_]

    w_qk = np.concatenate([w_qkv[:C] * scale, w_qkv[C:2 * C]], axis=0)
    W1 = np.ascontiguousarray(
        w_qk.reshape(12, 128, KT, 128).transpose(3, 0, 2, 1)).astype(NP_BF16)
    W2 = np.ascontiguousarray(
        w_qkv[2 * C:].reshape(C, KT, 128).transpose(2, 1, 0)).astype(NP_BF16)
    WP = np.ascontiguousarray(
        w_proj.reshape(KT, 128, KT, 128).transpose(3, 0, 2, 1)).astype(NP_BF16)
    ONES68 = np.ones((N, 1), NP_BF16)

    # x (1024, 68, 768) -> [core, p, k, b_local, t]
    xt = xp.reshape(NCORES, BL, N, KT, 128).transpose(0, 4, 3, 1, 2).astype(
        NP_BF16)

    shared = {"w1": W1, "w2": W2, "wp": WP, "ones68": ONES68}
    return xt, shared


def kernel(x, pe, mask, w_qkv, b_qkv, w_proj, b_proj):
    del mask  # zeros by problem spec
    xt, shared = _host_prep(x, pe, w_qkv, b_qkv, w_proj, b_proj)

    if "nc" not in _CACHE:
        _CACHE["nc"] = _build_nc()
    nc = _CACHE["nc"]

    in_maps = [dict(shared, xt=xt[c]) for c in range(NCORES)]
    res = run_bass_kernel_spmd(
        nc, in_maps, core_ids=list(range(NCORES)),
        **_CACHE.get("run_kwargs", {}),
    )
    _CACHE["last_result"] = res

    # outt [core, p, j, b, t] -> (1024, 68, 768)
    outt = np.stack([res.results[c]["outt"] for c in range(NCORES)])
    out = np.ascontiguousarray(
        outt.transpose(0, 3, 4, 2, 1).reshape(B_, N, C))
    return out


# revision 6
# speedup vs baseline: 1.0950x; 1.0950x over previous
"""Trainium2 Bass kernel for windowed multi-head attention (ClassicAttention).

Shapes (hardcoded per spec): x (1024, 68, 768), pe (128, 768), mask zeros.
Data-parallel over 8 NeuronCores on the leading window axis.

v2: bf16 matmul operands everywhere (fp32 HIGH mode ran at 4 passes/row),
pe folded into x on host, softmax normalization applied at the AT stage
(sum broadcast + wide reciprocal instead of a [1,408] DVE reciprocal).
"""

import os
import sys

for _p in (
    "/root/.axon_site",
    "/root/.axon_site/_ro/trn_rl_repo",
    "/root/.axon_site/_ro/pypackages",
    "/opt/trn_rl_repo",
):
    if os.path.isdir(_p) and _p not in sys.path:
        sys.path.append(_p)

import numpy as np
import ml_dtypes

import concourse.bass as bass
import concourse.mybir as mybir
import concourse.tile as tile
from concourse import bacc
from concourse.bass_utils import run_bass_kernel_spmd

F32 = mybir.dt.float32
BF16 = mybir.dt.bfloat16
NP_BF16 = np.dtype(ml_dtypes.bfloat16)
AFT = mybir.ActivationFunctionType

NCORES = 8
B_, N, C = 1024, 68, 768
H, HD = 12, 64
N_VTS = 4
KT = C // 128            # 6 contraction tiles of 128
BL = B_ // NCORES        # 128 windows per core
G = 4                    # windows per group (fp32 PSUM 4*68=272 <= 512)
NG = BL // G             # 32 groups
FD = G * N               # 272

_CACHE = {}


def _build_nc():
    nc = bacc.Bacc(trn_type="TRN2", target_bir_lowering=False, debug=False)

    xt_d = nc.dram_tensor("xt", [128, KT, BL, N], BF16, kind="ExternalInput")
    w1_d = nc.dram_tensor("w1", [128, 12, KT, 128], BF16, kind="ExternalInput")
    w2_d = nc.dram_tensor("w2", [128, KT, C], BF16, kind="ExternalInput")
    wp_d = nc.dram_tensor("wp", [128, KT, KT, 128], BF16, kind="ExternalInput")
    ones68_d = nc.dram_tensor("ones68", [N, 1], BF16, kind="ExternalInput")
    out_d = nc.dram_tensor("outt", [128, KT, BL, N], F32, kind="ExternalOutput")

    with tile.TileContext(nc) as tc:
        with (
            tc.tile_pool(name="wgt", bufs=1) as wp_pool,
            tc.tile_pool(name="xp", bufs=2) as xp,
            tc.tile_pool(name="qkp", bufs=2) as qkp,
            tc.tile_pool(name="vp", bufs=2) as vp,
            tc.tile_pool(name="esp", bufs=2) as esp,
            tc.tile_pool(name="rp", bufs=2) as rp,
            tc.tile_pool(name="atp", bufs=2) as atp,
            tc.tile_pool(name="pbig", bufs=2, space="PSUM") as pbig,
            tc.tile_pool(name="ppv", bufs=1, space="PSUM") as ppv,
            tc.tile_pool(name="psc", bufs=2, space="PSUM") as psc,
            tc.tile_pool(name="ps1", bufs=1, space="PSUM") as ps1p,
            tc.tile_pool(name="pav", bufs=1, space="PSUM") as pavp,
        ):
            W1s = wp_pool.tile([128, 12, KT, 128], BF16)
            W2s = wp_pool.tile([128, KT, C], BF16)
            WPs = wp_pool.tile([128, KT, KT, 128], BF16)
            ONES68s = wp_pool.tile([N, 1], BF16)
            nc.sync.dma_start(W1s[:], w1_d.ap())
            nc.sync.dma_start(W2s[:], w2_d.ap())
            nc.sync.dma_start(WPs[:], wp_d.ap())
            nc.sync.dma_start(ONES68s[:], ones68_d.ap())

            for g in range(NG):
                gsl = slice(G * g, G * (g + 1))
                XT = xp.tile([128, KT, G, N], BF16, tag="xt")
                nc.sync.dma_start(XT[:], xt_d.ap()[:, :, gsl, :])

                # ---- q,k in transposed layout: QKT[p, j, w, t] (j<6: q, j>=6: k)
                QKT = qkp.tile([128, 12, G, N], BF16, tag="qkt")
                for j in range(12):
                    pq = pbig.tile([128, FD], F32, tag="big")
                    for k in range(KT):
                        nc.tensor.matmul(
                            pq[:], W1s[:, j, k, :], XT[:, k, :, :],
                            start=(k == 0), stop=(k == KT - 1),
                        )
                    dst = QKT[:, j, :, :]
                    src = pq.rearrange("p (a b) -> p a b", a=G)
                    if j % 2 == 0:
                        nc.vector.tensor_copy(dst, src)
                    else:
                        nc.scalar.copy(dst, src)

                # ---- v in direct layout: V[t, w, o]
                V = vp.tile([N, G, C], BF16, tag="v")
                for w in range(G):
                    pv = ppv.tile([N, C], F32, tag="pv")
                    for hs in (slice(0, 512), slice(512, C)):
                        for k in range(KT):
                            nc.tensor.matmul(
                                pv[:, hs], XT[:, k, w, :], W2s[:, k, hs],
                                start=(k == 0), stop=(k == KT - 1),
                            )
                    nc.scalar.copy(V[:, w, :], pv[:])

                AT = atp.tile([128, KT, G, N], BF16, tag="at")
                wst = {}  # per-window attention state

                def stage_qk(w):
                    # ES slot s = 6*half + hh holds head h = 2*hh + half, so
                    # each PSUM bank sees a single PE row-group (HW hangs on
                    # mixed-row-group matmuls into one bank).
                    ES = esp.tile([N, H, N], BF16, tag="es", bufs=3)
                    for half in range(2):
                        sc = psc.tile([N, 6, N], F32, tag="sc")
                        p0 = 64 * half
                        for hh in range(6):
                            nc.tensor.matmul(
                                sc[:, hh, :],
                                QKT[p0:p0 + 64, 6 + hh, w, :],
                                QKT[p0:p0 + 64, hh, w, :],
                                start=True, stop=True, skip_group_check=True,
                            )
                        nc.scalar.activation(
                            ES[:, 6 * half:6 * half + 6, :], sc[:], AFT.Exp
                        )
                    S1 = rp.tile([1, 2, 408], F32, tag="s1s")
                    wst[w] = {"ES": ES, "S1": S1}

                def stage_sum(w, half):
                    # denominator: colsums of exp(S^T) for one half's 6 slots
                    s1 = ps1p.tile([1, 408], F32, tag="s1")
                    nc.tensor.matmul(
                        s1[:], ONES68s[:],
                        wst[w]["ES"][:, 6 * half:6 * half + 6, :],
                        start=True, stop=True,
                    )
                    nc.scalar.copy(wst[w]["S1"][:, half, :], s1[:])

                def stage_bcast(w):
                    # broadcast sums over partitions: RB[64a+d, s2, n] =
                    # S1[0, a, s2*68+n]  (head h = 2*s2 + a)
                    RB = rp.tile([128, KT, N], F32, tag="rb")
                    for a in range(2):
                        s1h = wst[w]["S1"][:, a, :]
                        src = bass.AP(
                            tensor=s1h.tensor, offset=s1h.offset,
                            ap=[list(s1h.ap[0]), [0, 64], [1, 408]],
                        )
                        nc.sync.dma_start(RB[64 * a:64 * a + 64, :, :], src)
                    RBr = rp.tile([128, KT, N], F32, tag="rbr", bufs=3)
                    nc.vector.reciprocal_approx_fast(RBr[:], RB[:])
                    wst[w]["RBr"] = RBr

                def stage_av(w):
                    # unnormalized attn @ v, normalized on PSUM->SBUF copy
                    st = wst.pop(w)
                    pa = pavp.tile([128, KT, N], F32, tag="pa")
                    for s in range(H):
                        h = 2 * (s % 6) + (s // 6)  # head held in ES slot s
                        nc.tensor.matmul(
                            pa[64 * (s // 6):64 * (s // 6) + 64, s % 6, :],
                            V[:, w, 64 * h:64 * (h + 1)],
                            st["ES"][:, s, :],
                            start=True, stop=True, skip_group_check=True,
                        )
                    nc.vector.tensor_mul(AT[:, :, w, :], pa[:], st["RBr"][:])

                # interleave keeping >=1 PE block between ps1/pav reuses
                stage_qk(0); stage_sum(0, 0)
                stage_qk(1); stage_sum(0, 1); stage_bcast(0)
                for w in range(2, G):
                    stage_qk(w); stage_sum(w - 1, 0); stage_av(w - 2)
                    stage_sum(w - 1, 1); stage_bcast(w - 1)
                stage_sum(G - 1, 0); stage_av(G - 2)
                stage_sum(G - 1, 1); stage_bcast(G - 1); stage_av(G - 1)

                # ---- proj: out^T[o, w, t] from PSUM via SBUF to DRAM
                for j in range(KT):
                    po = pbig.tile([128, FD], F32, tag="big")
                    for kt in range(KT):
                        nc.tensor.matmul(
                            po[:], WPs[:, j, kt, :], AT[:, kt, :, :],
                            start=(kt == 0), stop=(kt == KT - 1),
                        )
                    OT = atp.tile([128, G, N], F32, tag="ot")
                    src = po.rearrange("p (a b) -> p a b", a=G)
                    if j % 2 == 0:
                        nc.vector.tensor_copy(OT[:], src)
                    else:
                        nc.scalar.copy(OT[:], src)
                    nc.sync.dma_start(out_d.ap()[:, j, gsl, :], OT[:])

    nc.compile()
    return nc


def _host_prep(x, pe, w_qkv, b_qkv, w_proj, b_proj):
    f = np.float32
    x = np.asarray(x, f)
    pe = np.asarray(pe, f)
    w_qkv = np.asarray(w_qkv, f)
    w_proj = np.asarray(w_proj, f)

    scale = f(HD ** -0.5)
    n_ = N - N_VTS
    strt = pe.shape[0] // 2 - n_ // 2

    # fold pe into x (biases are zero by problem construction)
    xp = x.copy()
    xp[:, N_VTS:, :] += pe[strt:strt + n_]

    w_qk = np.concatenate([w_qkv[:C] * scale, w_qkv[C:2 * C]], axis=0)
    W1 = np.ascontiguousarray(
        w_qk.reshape(12, 128, KT, 128).transpose(3, 0, 2, 1)).astype(NP_BF16)
    W2 = np.ascontiguousarray(
        w_qkv[2 * C:].reshape(C, KT, 128).transpose(2, 1, 0)).astype(NP_BF16)
    WP = np.ascontiguousarray(
        w_proj.reshape(KT, 128, KT, 128).transpose(3, 0, 2, 1)).astype(NP_BF16)
    ONES68 = np.ones((N, 1), NP_BF16)

    # x (1024, 68, 768) -> [core, p, k, b_local, t]
    xt = xp.reshape(NCORES, BL, N, KT, 128).transpose(0, 4, 3, 1, 2).astype(
        NP_BF16)

    shared = {"w1": W1, "w2": W2, "wp": WP, "ones68": ONES68}
    return xt, shared


def kernel(x, pe, mask, w_qkv, b_qkv, w_proj, b_proj):
    del mask  # zeros by problem spec
    xt, shared = _host_prep(x, pe, w_qkv, b_qkv, w_proj, b_proj)

    if "nc" not in _CACHE:
        _CACHE["nc"] = _build_nc()
    nc = _CACHE["nc"]

    in_maps = [dict(shared, xt=xt[c]) for c in range(NCORES)]
    res = run_bass_kernel_spmd(
        nc, in_maps, core_ids=list(range(NCORES)),
        **_CACHE.get("run_kwargs", {}),
    )
    _CACHE["last_result"] = res

    # outt [core, p, j, b, t] -> (1024, 68, 768)
    outt = np.stack([res.results[c]["outt"] for c in range(NCORES)])
    out = np.ascontiguousarray(
        outt.transpose(0, 3, 4, 2, 1).reshape(B_, N, C))
    return out


# revision 11
# speedup vs baseline: 2.0088x; 1.8345x over previous
"""Trainium2 Bass kernel for windowed multi-head attention (ClassicAttention).

Shapes (hardcoded per spec): x (1024, 68, 768), pe (128, 768), mask zeros.
Data-parallel over 8 NeuronCores on the leading window axis.

v2: bf16 matmul operands everywhere (fp32 HIGH mode ran at 4 passes/row),
pe folded into x on host, softmax normalization applied at the AT stage
(sum broadcast + wide reciprocal instead of a [1,408] DVE reciprocal).
"""

import os
import sys

for _p in (
    "/root/.axon_site",
    "/root/.axon_site/_ro/trn_rl_repo",
    "/root/.axon_site/_ro/pypackages",
    "/opt/trn_rl_repo",
):
    if os.path.isdir(_p) and _p not in sys.path:
        sys.path.append(_p)

import numpy as np
import ml_dtypes

import concourse.bass as bass
import concourse.mybir as mybir
import concourse.tile as tile
from concourse import bacc
from concourse.bass_utils import run_bass_kernel_spmd

F32 = mybir.dt.float32
BF16 = mybir.dt.bfloat16
NP_BF16 = np.dtype(ml_dtypes.bfloat16)
AFT = mybir.ActivationFunctionType

NCORES = 8
B_, N, C = 1024, 68, 768
H, HD = 12, 64
N_VTS = 4
KT = C // 128            # 6 contraction tiles of 128
BL = B_ // NCORES        # 128 windows per core
G = 4                    # windows per group (fp32 PSUM 4*68=272 <= 512)
NG = BL // G             # 32 groups
FD = G * N               # 272

_CACHE = {}


def _build_nc():
    nc = bacc.Bacc(trn_type="TRN2", target_bir_lowering=False, debug=False)

    xt_d = nc.dram_tensor("xt", [128, KT, BL, N], BF16, kind="ExternalInput")
    w1_d = nc.dram_tensor("w1", [128, 12, KT, 128], BF16, kind="ExternalInput")
    w2_d = nc.dram_tensor("w2", [128, KT, C], BF16, kind="ExternalInput")
    wp_d = nc.dram_tensor("wp", [128, KT, KT, 128], BF16, kind="ExternalInput")
    ones68_d = nc.dram_tensor("ones68", [N, 64], BF16, kind="ExternalInput")
    out_d = nc.dram_tensor("outt", [128, KT, BL, N], F32, kind="ExternalOutput")

    with tile.TileContext(nc) as tc:
        with (
            tc.tile_pool(name="wgt", bufs=1) as wp_pool,
            tc.tile_pool(name="xp", bufs=2) as xp,
            tc.tile_pool(name="qkp", bufs=2) as qkp,
            tc.tile_pool(name="vp", bufs=2) as vp,
            tc.tile_pool(name="esp", bufs=2) as esp,
            tc.tile_pool(name="rp", bufs=2) as rp,
            tc.tile_pool(name="atp", bufs=2) as atp,
            tc.tile_pool(name="pbig", bufs=2, space="PSUM") as pbig,
            tc.tile_pool(name="ppv", bufs=1, space="PSUM") as ppv,
            tc.tile_pool(name="psc", bufs=2, space="PSUM") as psc,
            tc.tile_pool(name="ps1", bufs=1, space="PSUM") as ps1p,
            tc.tile_pool(name="pav", bufs=1, space="PSUM") as pavp,
        ):
            W1s = wp_pool.tile([128, 12, KT, 128], BF16)
            W2s = wp_pool.tile([128, KT, C], BF16)
            WPs = wp_pool.tile([128, KT, KT, 128], BF16)
            ONES68s = wp_pool.tile([N, 64], BF16)
            nc.sync.dma_start(W1s[:], w1_d.ap())
            nc.sync.dma_start(W2s[:], w2_d.ap())
            nc.sync.dma_start(WPs[:], wp_d.ap())
            nc.sync.dma_start(ONES68s[:], ones68_d.ap())

            for g in range(NG):
                gsl = slice(G * g, G * (g + 1))
                XT = xp.tile([128, KT, G, N], BF16, tag="xt")
                nc.sync.dma_start(XT[:], xt_d.ap()[:, :, gsl, :])

                # ---- q,k in transposed layout: QKT[p, j, w, t] (j<6: q, j>=6: k)
                QKT = qkp.tile([128, 12, G, N], BF16, tag="qkt")
                for j in range(12):
                    pq = pbig.tile([128, FD], F32, tag="big")
                    for k in range(KT):
                        nc.tensor.matmul(
                            pq[:], W1s[:, j, k, :], XT[:, k, :, :],
                            start=(k == 0), stop=(k == KT - 1),
                        )
                    dst = QKT[:, j, :, :]
                    src = pq.rearrange("p (a b) -> p a b", a=G)
                    if j % 2 == 0:
                        nc.vector.tensor_copy(dst, src)
                    else:
                        nc.scalar.copy(dst, src)

                # ---- v in direct layout: V[t, w, o]
                V = vp.tile([N, G, C], BF16, tag="v")
                for w in range(G):
                    pv = ppv.tile([N, C], F32, tag="pv")
                    for hs in (slice(0, 512), slice(512, C)):
                        for k in range(KT):
                            nc.tensor.matmul(
                                pv[:, hs], XT[:, k, w, :], W2s[:, k, hs],
                                start=(k == 0), stop=(k == KT - 1),
                            )
                    nc.scalar.copy(V[:, w, :], pv[:])

                AT = atp.tile([128, KT, G, N], BF16, tag="at")
                wst = {}  # per-window attention state

                def stage_qk(w):
                    # ES slot s = 6*half + hh holds head h = 2*hh + half, so
                    # each PSUM bank sees a single PE row-group (HW hangs on
                    # mixed-row-group matmuls into one bank).
                    ES = esp.tile([N, H, N], BF16, tag="es", bufs=3)
                    for half in range(2):
                        sc = psc.tile([N, 6, N], F32, tag="sc")
                        p0 = 64 * half
                        for hh in range(6):
                            nc.tensor.matmul(
                                sc[:, hh, :],
                                QKT[p0:p0 + 64, 6 + hh, w, :],
                                QKT[p0:p0 + 64, hh, w, :],
                                start=True, stop=True, skip_group_check=True,
                            )
                        nc.scalar.activation(
                            ES[:, 6 * half:6 * half + 6, :], sc[:], AFT.Exp
                        )
                    wst[w] = {"ES": ES}

                def stage_sum(w):
                    # denominators: colsums of exp(S^T), broadcast to all
                    # partitions by the 64-wide ones stationary itself:
                    # s1b[64a+d, s2, n] = sum_m ES[m, 6a+s2, n]  (head 2*s2+a,
                    # matching pa's layout).  Both halves into one bank, same
                    # row-group (like pa).
                    s1b = ps1p.tile([128, 6, N], F32, tag="s1b")
                    for half in range(2):
                        nc.tensor.matmul(
                            s1b[64 * half:64 * half + 64, :, :], ONES68s[:],
                            wst[w]["ES"][:, 6 * half:6 * half + 6, :],
                            start=True, stop=True, skip_group_check=True,
                        )
                    RBr = rp.tile([128, KT, N], F32, tag="rbr", bufs=3)
                    nc.vector.reciprocal_approx_fast(RBr[:], s1b[:])
                    wst[w]["RBr"] = RBr

                def stage_av(w):
                    # unnormalized attn @ v, normalized on PSUM->SBUF copy
                    st = wst.pop(w)
                    pa = pavp.tile([128, KT, N], F32, tag="pa")
                    for s in range(H):
                        h = 2 * (s % 6) + (s // 6)  # head held in ES slot s
                        nc.tensor.matmul(
                            pa[64 * (s // 6):64 * (s // 6) + 64, s % 6, :],
                            V[:, w, 64 * h:64 * (h + 1)],
                            st["ES"][:, s, :],
                            start=True, stop=True, skip_group_check=True,
                        )
                    nc.vector.tensor_mul(AT[:, :, w, :], pa[:], st["RBr"][:])

                # interleave keeping >=1 PE block between ps1b/pav reuses
                stage_qk(0); stage_qk(1); stage_sum(0)
                for w in range(2, G):
                    stage_qk(w); stage_av(w - 2); stage_sum(w - 1)
                stage_av(G - 2); stage_sum(G - 1); stage_av(G - 1)

                # ---- proj: out^T[o, w, t] from PSUM via SBUF to DRAM
                for j in range(KT):
                    po = pbig.tile([128, FD], F32, tag="big")
                    for kt in range(KT):
                        nc.tensor.matmul(
                            po[:], WPs[:, j, kt, :], AT[:, kt, :, :],
                            start=(kt == 0), stop=(kt == KT - 1),
                        )
                    OT = atp.tile([128, G, N], F32, tag="ot")
                    src = po.rearrange("p (a b) -> p a b", a=G)
                    if j % 2 == 0:
                        nc.vector.tensor_copy(OT[:], src)
                    else:
                        nc.scalar.copy(OT[:], src)
                    nc.sync.dma_start(out_d.ap()[:, j, gsl, :], OT[:])

    nc.compile()
    return nc


def _host_prep(x, pe, w_qkv, b_qkv, w_proj, b_proj):
    f = np.float32
    x = np.asarray(x, f)
    pe = np.asarray(pe, f)
    w_qkv = np.asarray(w_qkv, f)
    w_proj = np.asarray(w_proj, f)

    scale = f(HD ** -0.5)
    n_ = N - N_VTS
    strt = pe.shape[0] // 2 - n_ // 2

    # fold pe into x (biases are zero by problem construction)
    xp = x.copy()
    xp[:, N_VTS:, :] += pe[strt:strt + n_]

    w_qk = np.concatenate([w_qkv[:C] * scale, w_qkv[C:2 * C]], axis=0)
    W1 = np.ascontiguousarray(
        w_qk.reshape(12, 128, KT, 128).transpose(3, 0, 2, 1)).astype(NP_BF16)
    W2 = np.ascontiguousarray(
        w_qkv[2 * C:].reshape(C, KT, 128).transpose(2, 1, 0)).astype(NP_BF16)
    WP = np.ascontiguousarray(
        w_proj.reshape(KT, 128, KT, 128).transpose(3, 0, 2, 1)).astype(NP_BF16)
    ONES68 = np.ones((N, 64), NP_BF16)

    # x (1024, 68, 768) -> [core, p, k, b_local, t]
    xt = xp.reshape(NCORES, BL, N, KT, 128).transpose(0, 4, 3, 1, 2).astype(
        NP_BF16)

    shared = {"w1": W1, "w2": W2, "wp": WP, "ones68": ONES68}
    return xt, shared


def kernel(x, pe, mask, w_qkv, b_qkv, w_proj, b_proj):
    del mask  # zeros by problem spec
    xt, shared = _host_prep(x, pe, w_qkv, b_qkv, w_proj, b_proj)

    if "nc" not in _CACHE:
        _CACHE["nc"] = _build_nc()
    nc = _CACHE["nc"]

    in_maps = [dict(shared, xt=xt[c]) for c in range(NCORES)]
    res = run_bass_kernel_spmd(
        nc, in_maps, core_ids=list(range(NCORES)),
        **_CACHE.get("run_kwargs", {}),
    )
    _CACHE["last_result"] = res

    # outt [core, p, j, b, t] -> (1024, 68, 768)
    outt = np.stack([res.results[c]["outt"] for c in range(NCORES)])
    out = np.ascontiguousarray(
        outt.transpose(0, 3, 4, 2, 1).reshape(B_, N, C))
    return out
